# revision 12
# baseline (speedup 1.0000x reference)
"""Multi-head causal attention (B=2,S=2048,D=1024,H=16,dqk=dv=64) on 8 trn2
NeuronCores.

Sharding: tensor-parallel over heads (2 heads/core) for QKV+attention, then an
AllToAll flips to sequence-parallel (512 rows/core) for the output projection.

All matmuls run in bf16: this chip's PE clock governor caps sustained PE
utilization (throttle_activity_1 avg_util_limit = 0.5 -> K=4/8 pulse gating),
so wall time ~ PE cycle count. x is supplied host-side pre-transposed.

v2 structure (per core, both heads processed together):
  A. DMA x^T slices; Q^T/K^T/V^T = W.T @ x^T (feature-on-partition), bias on
     copy-out; V^T -> V via one [128,128] PE transpose per key chunk (both
     heads at once), with a ones column appended (softmax-denominator trick)
  B. flash attention in transposed-score layout, BOTH heads per chunk: the
     two K=64 score matmuls are row-tiled onto array halves (tile_position
     (0,0)/(64,0)) and run concurrently sharing the q-column stream -> the
     score pass costs ~half its v1 cycles.  exp(chunk k) on ACT overlaps
     P@V(chunk k-1) on PE.  Per-tile epilogue: [128,128] reciprocal + 8
     selector matmuls broadcast denominators; results DMA straight into the
     per-batch A2A slots (slot = 256-row quarter tile).
  C. one AllToAll per BATCH: b0's A2A overlaps b1's attention; b0's Wo
     matmuls overlap b1's A2A; only b1's Wo (16k cycles) is a serial tail.
Host: interleave the 8 [512,1024] row blocks (256 rows per batch per core).
"""

import numpy as np
import ml_dtypes

import bass_rust
import concourse.bass as bass
import concourse.mybir as mybir
import concourse.tile as tile
from concourse import bass_utils
from concourse.vector_clock import ScopedClock

# ---------------------------------------------------------------------------
# Workaround for this container's walrus build: it accepts at most ONE sync
# wait per instruction, but Tile emits several (tail drain + stage-1B waits).
# Split extra waits onto same-engine NoOps placed right before the instruction.
# ---------------------------------------------------------------------------

_waitsplit_cnt = [0]


def _patched_drain_and_barrier(self, tick_clock, wait_clock):
    nc = self.nc
    drain_inst = nc.sync.drain()
    wait_clock.add_sem_waits(
        drain_inst.ins, ScopedClock({None: tick_clock.global_clock})
    )
    si = drain_inst.ins.sync_info
    waits = list(si.on_wait) if si is not None else []
    if len(waits) > 1:
        drain_inst.ins.sync_info = bass_rust.SyncInfo(
            on_wait=[waits[0]], on_update=list(si.on_update)
        )
        for w in waits[1:]:
            d2 = nc.sync.drain()
            d2.ins.sync_info = bass_rust.SyncInfo(on_wait=[w], on_update=[])
    nc.all_engine_barrier()
    popped = nc._tile_sem_poison_stack.pop()
    assert popped is self._sem_poison
    nc.clear_and_free_semaphores(list(self.sems.allocated().values()))
    nc.all_engine_barrier()


tile.TileContext._drain_and_barrier = _patched_drain_and_barrier


def _split_multi_waits(nc):
    for f in nc.m.functions:
        for bb in f.blocks:
            insts = bb.instructions
            out = []
            dirty = False
            for inst in insts:
                si = inst.sync_info
                if si is not None and len(si.on_wait) > 1:
                    waits = list(si.on_wait)
                    for w in waits[:-1]:
                        nop = mybir.InstNoOp(
                            name=f"waitsplit_{_waitsplit_cnt[0]}", ins=[], outs=[]
                        )
                        _waitsplit_cnt[0] += 1
                        nop.engine = inst.engine
                        nop.sync_info = bass_rust.SyncInfo(on_wait=[w], on_update=[])
                        out.append(nop)
                    inst.sync_info = bass_rust.SyncInfo(
                        on_wait=[waits[-1]], on_update=list(si.on_update)
                    )
                    dirty = True
                out.append(inst)
            if dirty:
                bb.instructions = out


# ---------------------------------------------------------------------------
# Problem constants (hardcoded, self-contained)
# ---------------------------------------------------------------------------
B, S, D = 2, 2048, 1024
H, E = 16, 64           # heads, head dim
NCORES = 8
HL = H // NCORES        # heads per core = 2
BS = B * S              # 4096 flattened rows
ND = D // 128           # 8 d-chunks
ST = 512                # projection s-tile (rhs cols)
NST = BS // ST          # 8
TI = 512                # attention i-tile
NT_I = S // TI          # 4 per batch
TJ = 128                # key chunk
NJC = S // TJ           # 16 per batch
RQ = 256                # rows per A2A slot (quarter... half of a TI tile)
ROWS = BS // NCORES     # 512 output rows per core (256 per batch)

f32 = mybir.dt.float32
bf16 = mybir.dt.bfloat16
Exp = mybir.ActivationFunctionType.Exp
npbf16 = ml_dtypes.bfloat16

_built = [None]


def _build():
    nc = bass.Bass("TRN2", target_bir_lowering=False, debug=False,
                   num_devices=NCORES)

    xt_d = nc.dram_tensor("xt", (D, BS), bf16, kind="ExternalInput").ap()
    wq_d = nc.dram_tensor("wq", (D, 128), bf16, kind="ExternalInput").ap()
    wk_d = nc.dram_tensor("wk", (D, 128), bf16, kind="ExternalInput").ap()
    wv_d = nc.dram_tensor("wv", (D, 128), bf16, kind="ExternalInput").ap()
    bq_d = nc.dram_tensor("bq", (128, 1), f32, kind="ExternalInput").ap()
    bk_d = nc.dram_tensor("bk", (128, 1), f32, kind="ExternalInput").ap()
    bv_d = nc.dram_tensor("bv", (128, 1), f32, kind="ExternalInput").ap()
    wo_d = nc.dram_tensor("wo", (D, D), bf16, kind="ExternalInput").ap()
    bob_d = nc.dram_tensor("bob", (128, D), f32, kind="ExternalInput").ap()
    ident128_d = nc.dram_tensor("ident128", (128, 128), bf16,
                                kind="ExternalInput").ap()
    mask01_d = nc.dram_tensor("mask01", (128, 128), bf16,
                              kind="ExternalInput").ap()
    sel32_d = nc.dram_tensor("sel32", (128, 4 * E), bf16,
                             kind="ExternalInput").ap()

    out_d = nc.dram_tensor("out", (ROWS, D), f32, kind="ExternalOutput").ap()

    # one AllToAll per batch; slot s=2t+half carries [128 feats, 256 rows]
    a2a_in = [nc.dram_tensor(f"a2a_in{b}", (NCORES, 128, RQ), bf16,
                             kind="Internal").ap() for b in range(B)]
    a2a_out = [nc.dram_tensor(f"a2a_out{b}", (NCORES, 128, RQ), bf16,
                              kind="Internal").ap() for b in range(B)]

    with tile.TileContext(nc) as tc:
        with tc.tile_pool(name="persist", bufs=1) as pp:
            # big activation buffers, feature-on-partition, [2 heads x 64, B*S]
            xt_sb = pp.tile([128, ND, BS], bf16, tag="xt")
            qt = pp.tile([128, BS], bf16, tag="qt")
            kt = pp.tile([128, BS], bf16, tag="kt")
            vt = pp.tile([128, BS], bf16, tag="vt")
            # weights
            wq_sb = pp.tile([128, ND, 128], bf16, tag="wq")
            wk_sb = pp.tile([128, ND, 128], bf16, tag="wk")
            wv_sb = pp.tile([128, ND, 128], bf16, tag="wv")
            wo_sb = pp.tile([128, ND, D], bf16, tag="wo")
            bq_sb = pp.tile([128, 1], f32, tag="bq")
            bk_sb = pp.tile([128, 1], f32, tag="bk")
            bv_sb = pp.tile([128, 1], f32, tag="bv")
            bob_sb = pp.tile([128, D], f32, tag="bob")
            ident128_sb = pp.tile([128, 128], bf16, tag="ident128")
            mask01_sb = pp.tile([128, 128], bf16, tag="mask01")
            sel32_sb = pp.tile([128, 4 * E], bf16, tag="sel32")
            ones16 = pp.tile([128, NJC], bf16, tag="ones16")
            # per-(b,t,h) denominator staging rows (memset to 1.0 once; rows
            # 32c are overwritten each use, the rest stay 1.0 forever)
            dng_all = pp.tile([128, B * NT_I * HL, 128], f32, tag="dng")
            # A2A gather landing buffers [feat, src core, row]
            g_b = [pp.tile([128, NCORES, RQ], bf16, tag=f"g{b}",
                           name=f"g{b}") for b in range(B)]
            # V natural chunks + ones column: per (b, lh): [128 j, NJC, 65]
            vsb = [pp.tile([128, NJC, E + 1], bf16, tag=f"vsb{i}",
                           name=f"vsb{i}")
                   for i in range(B * HL)]

            # weights + x stream interleaved so the first projection can
            # start as soon as wq + x s-tile 0 land; wo is deferred to last
            xt_r = xt_d.rearrange("(c p) s -> p c s", p=128)

            def xslices(st):
                sl = slice(st * ST, (st + 1) * ST)
                nc.sync.dma_start(xt_sb[:, 0:4, sl], xt_r[:, 0:4, sl])
                nc.sync.dma_start(xt_sb[:, 4:8, sl], xt_r[:, 4:8, sl])

            nc.sync.dma_start(wq_sb[:], wq_d.rearrange("(c p) e -> p c e", p=128))
            xslices(0)
            nc.sync.dma_start(wk_sb[:], wk_d.rearrange("(c p) e -> p c e", p=128))
            nc.sync.dma_start(wv_sb[:], wv_d.rearrange("(c p) e -> p c e", p=128))
            nc.sync.dma_start(bq_sb[:], bq_d[:])
            nc.sync.dma_start(bk_sb[:], bk_d[:])
            nc.sync.dma_start(bv_sb[:], bv_d[:])
            xslices(1)
            nc.sync.dma_start(ident128_sb[:], ident128_d[:])
            xslices(2)
            nc.sync.dma_start(mask01_sb[:], mask01_d[:])
            nc.sync.dma_start(sel32_sb[:], sel32_d[:])
            xslices(3)
            for st in range(4, NST):
                xslices(st)
            nc.sync.dma_start(wo_sb[:], wo_d.rearrange("(c p) o -> p c o", p=128))
            nc.sync.dma_start(bob_sb[:], bob_d[:])
            nc.gpsimd.memset(ones16[:], 1.0)
            nc.gpsimd.memset(dng_all[:], 1.0)

            # ---------------- Phase A: QKV projections + V chunks -----------
            for b in range(B):
                for lh in range(HL):
                    with nc.allow_low_precision(reason="bf16 ones col"):
                        nc.vector.tensor_copy(vsb[b * HL + lh][:, :, E],
                                              ones16[:])
            # V^T->V transposes are delayed by one s-tile so the PE never
            # stalls on the DVE copy that materializes vt for that s-tile
            vjobs = []
            with tc.tile_pool(name="ptr", bufs=4, space="PSUM") as ptr_pool, \
                 tc.tile_pool(name="pproj", bufs=3, space="PSUM") as pproj_pool:

                def emit_vjobs(jobs):
                    # one [128,128] transpose flips a key-chunk of BOTH heads:
                    # V^T rows are (h0 dims 0-63 | h1 dims 64-127), so the
                    # transposed block is [128 keys, h0 V | h1 V]
                    for (bb_, jc) in jobs:
                        p_ = ptr_pool.tile([128, 128], bf16, tag="ptr")
                        nc.tensor.transpose(
                            p_[:],
                            vt[:, bb_ * S + jc * TJ: bb_ * S + (jc + 1) * TJ],
                            ident128_sb[:])
                        with nc.allow_low_precision(reason="bf16 V"):
                            for lh in range(HL):
                                nc.vector.tensor_copy(
                                    vsb[bb_ * HL + lh][:, jc, 0:E],
                                    p_[:, lh * E:(lh + 1) * E])

                for st in range(NST):
                    for wsb, bsb, dst in ((wq_sb, bq_sb, qt),
                                          (wk_sb, bk_sb, kt),
                                          (wv_sb, bv_sb, vt)):
                        pp_t = pproj_pool.tile([128, ST], f32, tag="pj")
                        for dc in range(ND):
                            nc.tensor.matmul(
                                pp_t[:], wsb[:, dc, :],
                                xt_sb[:, dc, st * ST:(st + 1) * ST],
                                start=(dc == 0), stop=(dc == ND - 1))
                        with nc.allow_low_precision(reason="bf16 proj"):
                            nc.vector.tensor_scalar_add(
                                dst[:, st * ST:(st + 1) * ST], pp_t[:], bsb[:])
                    emit_vjobs(vjobs)
                    bb_, jc0 = st // 4, 4 * (st % 4)
                    vjobs = [(bb_, jc) for jc in range(jc0, jc0 + 4)]
                emit_vjobs(vjobs)

            # ---------------- Phase B: flash attention, both heads ----------
            # Per chunk: the two K=64 score matmuls land on array row-halves
            # (tile_position inferred from kt slice base partition) and run
            # concurrently, sharing the q-column stream.  exp(chunk k)
            # overlaps P@V(chunk k-1).  PSUM budget: sc 4 + po 2 + aux 2 = 8.
            with tc.tile_pool(name="sc", bufs=4, space="PSUM") as sc_pool, \
                 tc.tile_pool(name="po", bufs=2, space="PSUM") as po_pool, \
                 tc.tile_pool(name="aux", bufs=2, space="PSUM") as aux_pool, \
                 tc.tile_pool(name="es", bufs=6) as es_pool, \
                 tc.tile_pool(name="osb", bufs=4) as osbp, \
                 tc.tile_pool(name="ost", bufs=8) as ostp, \
                 tc.tile_pool(name="rec", bufs=4) as recp, \
                 tc.tile_pool(name="ob", bufs=4) as ob_pool:

                def emit_pv(job):
                    # P^T @ [V|1]: K=128 keys, M=65 (out dims + denom row)
                    b, t, jc, ncols, coff, es_pair = job
                    for h in range(HL):
                        nc.tensor.matmul(
                            po_h[h][:, coff:TI],
                            vsb[b * HL + h][:, jc, :],
                            es_pair[h][:, 0:ncols],
                            start=(jc == 0), stop=(jc == 4 * t + 3))

                for b in range(B):
                    for t in range(NT_I):
                        po_h = [po_pool.tile([E + 1, TI], f32, tag="o",
                                             name=f"po{b}_{t}_{h}")
                                for h in range(HL)]
                        # chunk list: full chunks then column-shrunk diagonal
                        chunks = [(jc, TI, 0) for jc in range(4 * t)]
                        chunks += [(4 * t + ri, TI - 128 * ri, 128 * ri)
                                   for ri in range(4)]
                        pv_job = None
                        for (jc, ncols, coff) in chunks:
                            es_pair = []
                            for h in range(HL):
                                scp = sc_pool.tile([128, TI], f32, tag="s")
                                nc.tensor.matmul(
                                    scp[:, 0:ncols],
                                    kt[E * h:E * (h + 1),
                                       b * S + jc * TJ: b * S + (jc + 1) * TJ],
                                    qt[E * h:E * (h + 1),
                                       b * S + t * TI + coff:
                                       b * S + t * TI + coff + ncols],
                                    start=True, stop=True)
                                es = es_pool.tile([128, TI], bf16, tag="e")
                                with nc.allow_low_precision(reason="bf16 exp"):
                                    nc.scalar.activation(es[:, 0:ncols],
                                                         scp[:, 0:ncols], Exp,
                                                         scale=0.125)
                                    if coff or jc == 4 * t:
                                        # zero the causally-invalid upper
                                        # triangle of the leading 128 cols
                                        nc.vector.tensor_mul(es[:, 0:128],
                                                             es[:, 0:128],
                                                             mask01_sb[:])
                                es_pair.append(es)
                            if pv_job is not None:
                                emit_pv(pv_job)
                            pv_job = (b, t, jc, ncols, coff, es_pair)
                        emit_pv(pv_job)

                        # ---- per-tile epilogue: normalize + stage A2A slots
                        osb_h = []
                        recg_h = []
                        for h in range(HL):
                            osb = osbp.tile([E + 1, TI], bf16, tag="ob",
                                            name=f"osb{b}_{t}_{h}")
                            with nc.allow_low_precision(reason="bf16 O"):
                                nc.vector.tensor_copy(osb[:], po_h[h][:])
                            osb_h.append(osb)
                            dng = dng_all[:, HL * (4 * b + t) + h, :]
                            for c in range(4):
                                nc.vector.tensor_copy(
                                    dng[32 * c:32 * c + 1, :],
                                    osb[E:E + 1, 128 * c:128 * (c + 1)])
                            recg = recp.tile([128, 128], bf16, tag="rec",
                                             name=f"rec{b}_{t}_{h}")
                            with nc.allow_low_precision(reason="denom"):
                                nc.vector.reciprocal(recg[:], dng)
                            recg_h.append(recg)
                        pb = aux_pool.tile([128, TI], f32, tag="pw")
                        for h in range(HL):
                            for c in range(4):
                                nc.tensor.matmul(
                                    pb[E * h:E * (h + 1),
                                       128 * c:128 * (c + 1)],
                                    sel32_sb[:, c * E:(c + 1) * E],
                                    recg_h[h][:], start=True, stop=True)
                        for h in range(HL):
                            ost = ostp.tile([E, TI], bf16, tag="ost")
                            with nc.allow_low_precision(reason="bf16 ost"):
                                nc.vector.tensor_mul(
                                    ost[:], osb_h[h][0:E, :],
                                    pb[E * h:E * (h + 1), :])
                            for half in range(2):
                                nc.sync.dma_start(
                                    a2a_in[b][2 * t + half,
                                              E * h:E * (h + 1), :],
                                    ost[:, RQ * half:RQ * (half + 1)])

                    nc.gpsimd.collective_compute(
                        "AllToAll", mybir.AluOpType.bypass,
                        replica_groups=[list(range(NCORES))],
                        ins=[a2a_in[b][:]], outs=[a2a_out[b][:]])
                    if b == 0:
                        # stage b0's A2A results into SBUF during b1 attention
                        nc.sync.dma_start(
                            g_b[0][:], a2a_out[0].rearrange("f e r -> e f r"))

                # ------- Phase C: Wo.  b0's matmuls run while b1's AllToAll
                # is still in flight; only b1's half is a serial tail.  The
                # b1 gather DMA is emitted after b0's Wo so b0's output DMAs
                # don't queue behind its wait on the collective.
                for b in range(B):
                    if b == 1:
                        nc.sync.dma_start(
                            g_b[1][:],
                            a2a_out[1].rearrange("f e r -> e f r"))
                    for rb in range(RQ // 128):
                        for ot in range(D // 512):
                            pw = aux_pool.tile([128, 512], f32, tag="pw")
                            for fi in range(NCORES):
                                nc.tensor.matmul(
                                    pw[:],
                                    g_b[b][:, fi, rb * 128:(rb + 1) * 128],
                                    wo_sb[:, fi, ot * 512:(ot + 1) * 512],
                                    start=(fi == 0), stop=(fi == NCORES - 1))
                            ob = ob_pool.tile([128, 512], f32, tag="obo")
                            nc.vector.tensor_add(
                                ob[:], pw[:],
                                bob_sb[:, ot * 512:(ot + 1) * 512])
                            nc.sync.dma_start(
                                out_d[b * RQ + rb * 128:
                                      b * RQ + (rb + 1) * 128,
                                      ot * 512:(ot + 1) * 512],
                                ob[:])

    _split_multi_waits(nc)
    return nc


def _get_nc():
    if _built[0] is None:
        _built[0] = _build()
    return _built[0]


def _host_inputs(x, Wq, bq, Wk, bk, Wv, bv, Wo, bo):
    xT = np.ascontiguousarray(
        np.asarray(x, dtype=np.float32).reshape(BS, D).T).astype(npbf16)
    Wq = np.asarray(Wq, dtype=np.float32)
    Wk = np.asarray(Wk, dtype=np.float32)
    Wv = np.asarray(Wv, dtype=np.float32)
    bq = np.asarray(bq, dtype=np.float32)
    bk = np.asarray(bk, dtype=np.float32)
    bv = np.asarray(bv, dtype=np.float32)
    Wo = np.ascontiguousarray(np.asarray(Wo, dtype=np.float32)).astype(npbf16)
    bo = np.asarray(bo, dtype=np.float32)

    ident128 = np.eye(128).astype(npbf16)
    jj = np.arange(128, dtype=np.int64)[:, None]
    ii = np.arange(128, dtype=np.int64)[None, :]
    mask01 = (jj <= ii).astype(npbf16)
    sel32 = np.zeros((128, 4 * E), dtype=npbf16)
    for c in range(4):
        sel32[32 * c, c * E:(c + 1) * E] = 1.0
    bob = np.tile(bo[None, :], (128, 1)).astype(np.float32)

    in_maps = []
    for c in range(NCORES):
        hs = slice(HL * c, HL * (c + 1))
        in_maps.append({
            "xt": xT,
            "wq": np.ascontiguousarray(
                Wq[hs].transpose(1, 0, 2).reshape(D, 128)).astype(npbf16),
            "wk": np.ascontiguousarray(
                Wk[hs].transpose(1, 0, 2).reshape(D, 128)).astype(npbf16),
            "wv": np.ascontiguousarray(
                Wv[hs].transpose(1, 0, 2).reshape(D, 128)).astype(npbf16),
            "bq": np.ascontiguousarray(bq[hs].reshape(128, 1)),
            "bk": np.ascontiguousarray(bk[hs].reshape(128, 1)),
            "bv": np.ascontiguousarray(bv[hs].reshape(128, 1)),
            "wo": Wo,
            "bob": bob,
            "ident128": ident128,
            "mask01": mask01,
            "sel32": sel32,
        })
    return in_maps


def kernel(x, Wq, bq, Wk, bk, Wv, bv, Wo, bo, _trace=False, _tmpdir=None):
    nc = _get_nc()
    in_maps = _host_inputs(x, Wq, bq, Wk, bk, Wv, bv, Wo, bo)
    res = bass_utils.run_bass_kernel_spmd(
        nc, in_maps, core_ids=list(range(NCORES)),
        trace=_trace, tmpdir=_tmpdir)
    # core c returns [512, 1024]: rows 0:256 = batch0, 256:512 = batch1 of
    # global row block 512*(c//2) + 256*(c%2)
    out = np.empty((B, S, D), dtype=np.float32)
    for c in range(NCORES):
        gr = 512 * (c // 2) + 256 * (c % 2)
        blk = res.results[c]["out"]
        for b in range(B):
            out[b, gr:gr + RQ, :] = blk[b * RQ:(b + 1) * RQ, :]
    kernel.last_exec_time_ns = res.exec_time_ns
    kernel.last_results = res
    return out


kernel.last_exec_time_ns = None
kernel.last_results = None
